# revision 12
# baseline (speedup 1.0000x reference)
"""Trainium2 Bass kernel for nn_ChannelLinearCombo.

out[b, o, h, w] = sum_c x[b, c, h, w] * weights[o, c]

Strategy: data-parallel over batch B=32 across 8 cores (4 batches/core).
Per core the problem is a GEMM per batch: out[b] (O=512, HW=3136) =
W (512, 256) @ x[b] (256, 3136), computed on the tensor engine with
C split into 2 K-tiles of 128 (PSUM accumulation), O split into 4
M-tiles of 128 partitions, and HW split into 7 N-tiles of 448 columns
(448 fp32 <= 512 per PSUM bank).  Weights (transposed on host to (C, O))
stay resident in SBUF; x tiles stream through double-buffered pools.

Numeric modes:
  fp32   - native fp32 matmul (4 cycles/row on the PE)
  fp32r  - hardware relaxed-precision fp32 matmul (1 cycle/row)
  split3 - x and W split on host into bf16 hi+lo; out = Wh@xh + Wh@xl
           + Wl@xh accumulated in PSUM (3 bf16 passes, 3 cycles/row,
           ~4e-6 relative error)
"""

import numpy as np
import ml_dtypes

import concourse.bacc as bacc
import concourse.mybir as mybir
import concourse.tile as tile
import concourse.bass_utils as bass_utils

B, C, O, H, W = 32, 256, 512, 56, 56
HW = H * W                      # 3136
NCORES = 8
BPC = B // NCORES               # 4 batches per core
NT = 448                        # N-tile (columns of the moving operand)
NTJ = HW // NT                  # 7 tiles per batch
KT = C // 128                   # 2 contraction tiles
MT = O // 128                   # 4 output-channel tiles

FP32 = mybir.dt.float32
FP32R = mybir.dt.float32r
BF16 = mybir.dt.bfloat16

MODE = "split3"


def _new_nc():
    return bacc.Bacc(
        "TRN2",
        target_bir_lowering=False,
        debug=False,
        num_devices=NCORES,
    )


def _build_fp32(mm_dt):
    """fp32 / fp32r single-pass kernel."""
    nc = _new_nc()
    x = nc.dram_tensor("x", [BPC, C, HW], mm_dt, kind="ExternalInput").ap()
    wT = nc.dram_tensor("wT", [C, O], mm_dt, kind="ExternalInput").ap()
    out = nc.dram_tensor("out", [BPC, O, HW], FP32, kind="ExternalOutput").ap()

    with tile.TileContext(nc) as tc:
        with (
            tc.tile_pool(name="wpool", bufs=1) as wpool,
            tc.tile_pool(name="xpool", bufs=4) as xpool,
            tc.tile_pool(name="opool", bufs=8) as opool,
            tc.tile_pool(name="ppool", bufs=8, space="PSUM") as ppool,
        ):
            w_sb = []
            for k in range(KT):
                wt = wpool.tile([128, O], mm_dt, tag=f"w{k}", name=f"w{k}")
                nc.sync.dma_start(wt[:], wT[k * 128:(k + 1) * 128, :])
                w_sb.append(wt)

            for b in range(BPC):
                for j in range(NTJ):
                    js = slice(j * NT, (j + 1) * NT)
                    xts = []
                    for k in range(KT):
                        xt = xpool.tile([128, NT], mm_dt, tag="x", name="xt")
                        nc.sync.dma_start(xt[:], x[b, k * 128:(k + 1) * 128, js])
                        xts.append(xt)
                    for m in range(MT):
                        ms = slice(m * 128, (m + 1) * 128)
                        pt = ppool.tile([128, NT], FP32, tag="p", name="pt")
                        for k in range(KT):
                            nc.tensor.matmul(
                                pt[:], w_sb[k][:, ms], xts[k][:],
                                start=(k == 0), stop=(k == KT - 1),
                            )
                        ot = opool.tile([128, NT], FP32, tag="o", name="ot")
                        nc.vector.tensor_copy(ot[:], pt[:])
                        nc.sync.dma_start(out[b, ms, js], ot[:])
    nc.compile()
    return nc


def _build_split3():
    """bf16 hi/lo 3-pass kernel: out = Wh@xh + Wh@xl + Wl@xh."""
    nc = _new_nc()
    xh = nc.dram_tensor("xh", [BPC, C, HW], BF16, kind="ExternalInput").ap()
    xl = nc.dram_tensor("xl", [BPC, C, HW], BF16, kind="ExternalInput").ap()
    wh = nc.dram_tensor("wh", [C, O], BF16, kind="ExternalInput").ap()
    wl = nc.dram_tensor("wl", [C, O], BF16, kind="ExternalInput").ap()
    out = nc.dram_tensor("out", [BPC, O, HW], FP32, kind="ExternalOutput").ap()

    with tile.TileContext(nc) as tc:
        with (
            tc.tile_pool(name="wpool", bufs=1) as wpool,
            tc.tile_pool(name="xpool", bufs=8) as xpool,
            tc.tile_pool(name="opool", bufs=8) as opool,
            tc.tile_pool(name="ppool", bufs=8, space="PSUM") as ppool,
        ):
            wh_sb, wl_sb = [], []
            for k in range(KT):
                wht = wpool.tile([128, O], BF16, tag=f"wh{k}", name=f"wh{k}")
                nc.sync.dma_start(wht[:], wh[k * 128:(k + 1) * 128, :])
                wh_sb.append(wht)
                wlt = wpool.tile([128, O], BF16, tag=f"wl{k}", name=f"wl{k}")
                nc.sync.dma_start(wlt[:], wl[k * 128:(k + 1) * 128, :])
                wl_sb.append(wlt)

            for b in range(BPC):
                for j in range(NTJ):
                    js = slice(j * NT, (j + 1) * NT)
                    xh_t, xl_t = [], []
                    for k in range(KT):
                        ks = slice(k * 128, (k + 1) * 128)
                        xht = xpool.tile([128, NT], BF16, tag="xh", name="xht")
                        nc.sync.dma_start(xht[:], xh[b, ks, js])
                        xh_t.append(xht)
                        xlt = xpool.tile([128, NT], BF16, tag="xl", name="xlt")
                        nc.sync.dma_start(xlt[:], xl[b, ks, js])
                        xl_t.append(xlt)
                    for m in range(MT):
                        ms = slice(m * 128, (m + 1) * 128)
                        pt = ppool.tile([128, NT], FP32, tag="p", name="pt")
                        passes = []
                        for k in range(KT):
                            passes += [
                                (wh_sb[k][:, ms], xh_t[k][:]),
                                (wh_sb[k][:, ms], xl_t[k][:]),
                                (wl_sb[k][:, ms], xh_t[k][:]),
                            ]
                        for i, (wop, xop) in enumerate(passes):
                            nc.tensor.matmul(
                                pt[:], wop, xop,
                                start=(i == 0), stop=(i == len(passes) - 1),
                            )
                        ot = opool.tile([128, NT], FP32, tag="o", name="ot")
                        nc.vector.tensor_copy(ot[:], pt[:])
                        nc.sync.dma_start(out[b, ms, js], ot[:])
    nc.compile()
    return nc


def _build_fp32_wide(mm_dt):
    """fp32 / fp32r single-pass kernel with full-row (HW-wide) DMA.

    x loads and out stores move whole [128, 3136] rows (12.5 KB/partition
    contiguous bursts) instead of per-448-column slices, maximizing HBM
    efficiency; matmuls slice the resident SBUF rows.
    """
    nc = _new_nc()
    x = nc.dram_tensor("x", [BPC, C, HW], mm_dt, kind="ExternalInput").ap()
    wT = nc.dram_tensor("wT", [C, O], mm_dt, kind="ExternalInput").ap()
    out = nc.dram_tensor("out", [BPC, O, HW], FP32, kind="ExternalOutput").ap()

    with tile.TileContext(nc) as tc:
        with (
            tc.tile_pool(name="wpool", bufs=1) as wpool,
            tc.tile_pool(name="xpool", bufs=6) as xpool,
            tc.tile_pool(name="opool", bufs=6) as opool,
            tc.tile_pool(name="ppool", bufs=8, space="PSUM") as ppool,
        ):
            w_sb = []
            for k in range(KT):
                wt = wpool.tile([128, O], mm_dt, tag=f"w{k}", name=f"w{k}")
                nc.sync.dma_start(wt[:], wT[k * 128:(k + 1) * 128, :])
                w_sb.append(wt)

            for b in range(BPC):
                xts = []
                for k in range(KT):
                    xt = xpool.tile([128, HW], mm_dt, tag="x", name="xt")
                    nc.sync.dma_start(xt[:], x[b, k * 128:(k + 1) * 128, :])
                    xts.append(xt)
                for m in range(MT):
                    ms = slice(m * 128, (m + 1) * 128)
                    ot = opool.tile([128, HW], FP32, tag="o", name="ot")
                    for n in range(NTJ):
                        ns = slice(n * NT, (n + 1) * NT)
                        pt = ppool.tile([128, NT], FP32, tag="p", name="pt")
                        for k in range(KT):
                            nc.tensor.matmul(
                                pt[:], w_sb[k][:, ms], xts[k][:, ns],
                                start=(k == 0), stop=(k == KT - 1),
                            )
                        nc.vector.tensor_copy(ot[:, ns], pt[:])
                    nc.sync.dma_start(out[b, ms, :], ot[:])
    nc.compile()
    return nc


def _build_split3_wide():
    """bf16 hi/lo 3-pass kernel with full-row (HW-wide) DMA."""
    nc = _new_nc()
    xh = nc.dram_tensor("xh", [BPC, C, HW], BF16, kind="ExternalInput").ap()
    xl = nc.dram_tensor("xl", [BPC, C, HW], BF16, kind="ExternalInput").ap()
    wh = nc.dram_tensor("wh", [C, O], BF16, kind="ExternalInput").ap()
    wl = nc.dram_tensor("wl", [C, O], BF16, kind="ExternalInput").ap()
    out = nc.dram_tensor("out", [BPC, O, HW], FP32, kind="ExternalOutput").ap()

    with tile.TileContext(nc) as tc:
        with (
            tc.tile_pool(name="wpool", bufs=1) as wpool,
            tc.tile_pool(name="xpool", bufs=6) as xpool,
            tc.tile_pool(name="opool", bufs=6) as opool,
            tc.tile_pool(name="ppool", bufs=8, space="PSUM") as ppool,
        ):
            wh_sb, wl_sb = [], []
            for k in range(KT):
                wht = wpool.tile([128, O], BF16, tag=f"wh{k}", name=f"wh{k}")
                nc.sync.dma_start(wht[:], wh[k * 128:(k + 1) * 128, :])
                wh_sb.append(wht)
                wlt = wpool.tile([128, O], BF16, tag=f"wl{k}", name=f"wl{k}")
                nc.sync.dma_start(wlt[:], wl[k * 128:(k + 1) * 128, :])
                wl_sb.append(wlt)

            for b in range(BPC):
                xh_t, xl_t = [], []
                for k in range(KT):
                    ks = slice(k * 128, (k + 1) * 128)
                    xht = xpool.tile([128, HW], BF16, tag="xh", name="xht")
                    nc.sync.dma_start(xht[:], xh[b, ks, :])
                    xh_t.append(xht)
                    xlt = xpool.tile([128, HW], BF16, tag="xl", name="xlt")
                    nc.sync.dma_start(xlt[:], xl[b, ks, :])
                    xl_t.append(xlt)
                for m in range(MT):
                    ms = slice(m * 128, (m + 1) * 128)
                    ot = opool.tile([128, HW], FP32, tag="o", name="ot")
                    for n in range(NTJ):
                        ns = slice(n * NT, (n + 1) * NT)
                        pt = ppool.tile([128, NT], FP32, tag="p", name="pt")
                        passes = []
                        for k in range(KT):
                            passes += [
                                (wh_sb[k][:, ms], xh_t[k][:, ns]),
                                (wh_sb[k][:, ms], xl_t[k][:, ns]),
                                (wl_sb[k][:, ms], xh_t[k][:, ns]),
                            ]
                        for i, (wop, xop) in enumerate(passes):
                            nc.tensor.matmul(
                                pt[:], wop, xop,
                                start=(i == 0), stop=(i == len(passes) - 1),
                            )
                        nc.vector.tensor_copy(ot[:, ns], pt[:])
                    nc.sync.dma_start(out[b, ms, :], ot[:])
    nc.compile()
    return nc


def _build_split3_v3():
    """split3_wide plus: PE warm-up matmuls during the initial x loads
    (beats the HAM cold clock), half-row-chunked x loads (first matmul
    group starts ~2.5us in instead of ~9us), and per-n-tile stores on the
    last batch (shrinks the final store tail)."""
    nc = _new_nc()
    xh = nc.dram_tensor("xh", [BPC, C, HW], BF16, kind="ExternalInput").ap()
    xl = nc.dram_tensor("xl", [BPC, C, HW], BF16, kind="ExternalInput").ap()
    wh = nc.dram_tensor("wh", [C, O], BF16, kind="ExternalInput").ap()
    wl = nc.dram_tensor("wl", [C, O], BF16, kind="ExternalInput").ap()
    out = nc.dram_tensor("out", [BPC, O, HW], FP32, kind="ExternalOutput").ap()

    CH0 = 4 * NT                  # 1792 cols: n-tiles 0..3
    CH1 = HW - CH0                # 1344 cols: n-tiles 4..6

    with tile.TileContext(nc) as tc:
        with (
            tc.tile_pool(name="wpool", bufs=1) as wpool,
            tc.tile_pool(name="xpool", bufs=8) as xpool,
            tc.tile_pool(name="opool", bufs=8) as opool,
            tc.tile_pool(name="ppool", bufs=7, space="PSUM") as ppool,
        ):
            wh_sb, wl_sb = [], []
            for k in range(KT):
                wht = wpool.tile([128, O], BF16, tag=f"wh{k}", name=f"wh{k}")
                nc.sync.dma_start(wht[:], wh[k * 128:(k + 1) * 128, :])
                wh_sb.append(wht)
                wlt = wpool.tile([128, O], BF16, tag=f"wl{k}", name=f"wl{k}")
                nc.sync.dma_start(wlt[:], wl[k * 128:(k + 1) * 128, :])
                wl_sb.append(wlt)

            # ~4.5us of dummy matmuls on the (tiny, already-loaded) weight
            # tile while the first x rows stream in: HAM sees a busy PE and
            # lifts the 1.2 GHz cold clock before real work starts.
            wp = ppool.tile([128, NT], FP32, tag="pw", name="wp", bufs=1)
            for _ in range(24):
                nc.tensor.matmul(
                    wp[:], wh_sb[0][:, :128], wh_sb[0][:, :NT],
                    start=True, stop=True,
                )

            for b in range(BPC):
                xt = {}
                for c, (c0, cw) in enumerate(((0, CH0), (CH0, CH1))):
                    for k in range(KT):
                        ks = slice(k * 128, (k + 1) * 128)
                        cs = slice(c0, c0 + cw)
                        xht = xpool.tile([128, cw], BF16, tag="xh",
                                         name="xht", padded_shape=[128, CH0])
                        nc.sync.dma_start(xht[:], xh[b, ks, cs])
                        xt["h", k, c] = xht
                        xlt = xpool.tile([128, cw], BF16, tag="xl",
                                         name="xlt", padded_shape=[128, CH0])
                        nc.sync.dma_start(xlt[:], xl[b, ks, cs])
                        xt["l", k, c] = xlt
                for m in range(MT):
                    ms = slice(m * 128, (m + 1) * 128)
                    ot = opool.tile([128, HW], FP32, tag="o", name="ot")
                    for n in range(NTJ):
                        c = 0 if n < 4 else 1
                        ns = slice((n - 4 * c) * NT, (n - 4 * c + 1) * NT)
                        os_ = slice(n * NT, (n + 1) * NT)
                        pt = ppool.tile([128, NT], FP32, tag="p", name="pt")
                        passes = []
                        for k in range(KT):
                            passes += [
                                (wh_sb[k][:, ms], xt["h", k, c][:, ns]),
                                (wh_sb[k][:, ms], xt["l", k, c][:, ns]),
                                (wl_sb[k][:, ms], xt["h", k, c][:, ns]),
                            ]
                        for i, (wop, xop) in enumerate(passes):
                            nc.tensor.matmul(
                                pt[:], wop, xop,
                                start=(i == 0), stop=(i == len(passes) - 1),
                            )
                        nc.vector.tensor_copy(ot[:, os_], pt[:])
                        if b == BPC - 1:
                            nc.sync.dma_start(out[b, ms, os_], ot[:, os_])
                    if b < BPC - 1:
                        nc.sync.dma_start(out[b, ms, :], ot[:])
    nc.compile()
    return nc


_nc_cache = {}

_BUILDERS = {
    "fp32": lambda: _build_fp32(FP32),
    "fp32r": lambda: _build_fp32(FP32R),
    "split3": _build_split3,
    "fp32_wide": lambda: _build_fp32_wide(FP32),
    "fp32r_wide": lambda: _build_fp32_wide(FP32R),
    "split3_wide": _build_split3_wide,
    "split3_v3": _build_split3_v3,
}


def _get_nc(mode):
    if mode not in _nc_cache:
        _nc_cache[mode] = _BUILDERS[mode]()
    return _nc_cache[mode]


def kernel(x, weights, mode=None):
    mode = mode or MODE
    x = np.ascontiguousarray(np.asarray(x, dtype=np.float32))
    weights = np.asarray(weights, dtype=np.float32)
    assert x.shape == (B, C, H, W)
    assert weights.shape == (O, C)

    x_sh = x.reshape(NCORES, BPC, C, HW)
    wT = np.ascontiguousarray(weights.T)          # (C, O)

    nc = _get_nc(mode)

    if mode.startswith("split3"):
        bf16 = ml_dtypes.bfloat16
        xh = x_sh.astype(bf16)
        xl = (x_sh - xh.astype(np.float32)).astype(bf16)
        wh = wT.astype(bf16)
        wl = (wT - wh.astype(np.float32)).astype(bf16)
        in_maps = [
            {"xh": xh[i], "xl": xl[i], "wh": wh, "wl": wl}
            for i in range(NCORES)
        ]
    else:
        in_maps = [{"x": x_sh[i], "wT": wT} for i in range(NCORES)]

    res = bass_utils.run_bass_kernel_spmd(nc, in_maps, core_ids=list(range(NCORES)))
    kernel._last_results = res

    out = np.empty((B, O, H, W), dtype=np.float32)
    for i in range(NCORES):
        out[i * BPC:(i + 1) * BPC] = res.results[i]["out"].reshape(BPC, O, H, W)
    return out
